# revision 1
# baseline (speedup 1.0000x reference)
"""ConsecutiveLoss (L1) Trainium2 kernel.

Reference semantics (per full input x [4096, 8192] f32):
    rl[i]     = count_nonzero(x[i, :])
    per_row_i = sum_{j=0}^{8190} |x[i,j+1]-x[i,j]| * (j < rl[i]-1) / rl[i]
    out       = sum_{i>=1} per_row_i / 4096

Sharding: 4096 rows split across 8 NeuronCores (512 rows each). Each core
computes per-row losses for its rows; host gathers and does the final
(4095-element) reduction.

Per-core kernel: 4 tiles of [128 rows x 8192], two column-chunks each for
pipelining. Per tile:
  - DMA the tile into SBUF (two 2 MiB chunks)
  - rl: DVE tensor_scalar(not_equal)+accum per chunk (2x single-src mode)
  - sub: DVE tensor_tensor(subtract), bf16 out
  - abs: ACT activation(Abs) bf16
  - masked row-sum: DVE scalar_tensor_tensor
        (iota16 is_lt rl-1) mult |d|, accum_out -> rowsum
    with iota int16 + |d| bf16 (16-bit streams for a shot at 2x mode)
  - per-row loss = (rs0+rs1) * 1/rl; collected in SBUF, one DMA out.

This walrus build accepts only ONE sync wait per ISA instruction; TileContext
emits multi-wait instructions (stage-1B consumers + the tail drain). Both are
patched below by splitting waits onto single-wait NoOp/Drain carriers.
"""

import os
from operator import add

import numpy as np

import concourse.bass as bass
import concourse.mybir as mybir
import concourse.tile as tile
from concourse.bass_utils import run_bass_kernel_spmd

# --- workaround: single-sync-wait-per-instruction walrus ---
_ORIG_DRAIN_AND_BARRIER = tile.TileContext._drain_and_barrier


def _split_drain_and_barrier(self, tick_clock, wait_clock):
    from concourse.tile import ScopedClock

    drain_inst = self.nc.sync.drain()
    wait_clock.add_sem_waits(
        drain_inst.ins, ScopedClock({None: tick_clock.global_clock})
    )
    si = drain_inst.ins.sync_info
    waits = list(si.on_wait) if si is not None and si.on_wait else []
    if len(waits) > 1:
        ups = list(si.on_update) if si.on_update else []
        drain_inst.ins.sync_info = mybir.SyncInfo(on_wait=[waits[0]], on_update=ups)
        for w in waits[1:]:
            extra = self.nc.sync.drain()
            extra.ins.sync_info = mybir.SyncInfo(on_wait=[w], on_update=[])

    self.nc.all_engine_barrier()
    assert self.sems is not None
    popped = self.nc._tile_sem_poison_stack.pop()
    assert popped is self._sem_poison
    self.nc.clear_and_free_semaphores(list(self.sems.allocated().values()))
    self.nc.all_engine_barrier()


tile.TileContext._drain_and_barrier = _split_drain_and_barrier

_ORIG_COMMIT = tile.TileContext._commit_instruction


def _split_commit(self, inst, lazy_reg_writes: bool = True):
    si = getattr(inst, "sync_info", None)
    if (
        si is not None
        and si.on_wait
        and len(si.on_wait) > 1
        and inst.engine != mybir.EngineType.Unassigned
    ):
        waits = list(si.on_wait)
        ups = list(si.on_update) if si.on_update else []
        for w in waits[:-1]:
            nop = mybir.InstNoOp(
                name=self.nc.get_next_instruction_name(),
                sync_info=mybir.SyncInfo(on_wait=[w], on_update=[]),
                bass_nofuse=True,
                engine=inst.engine,
                text_hint="wait_split",
            )
            _ORIG_COMMIT(self, nop, lazy_reg_writes=False)
        inst.sync_info = mybir.SyncInfo(on_wait=[waits[-1]], on_update=ups)
    return _ORIG_COMMIT(self, inst, lazy_reg_writes)


tile.TileContext._commit_instruction = _split_commit


def _audit_multi_waits(nc) -> list[str]:
    bad = []
    for name, ins in nc.inst_map.items():
        si = getattr(ins, "sync_info", None)
        if si is not None and si.on_wait and len(si.on_wait) > 1:
            bad.append(f"{type(ins).__name__} {name} {ins.engine} x{len(si.on_wait)}")
    return bad


N_CORES = 8
ROWS, COLS = 4096, 8192
SH_ROWS = ROWS // N_CORES  # 512 rows per core
P = 128                    # SBUF partitions
N_TILES = SH_ROWS // P     # 4 tiles per core
D = COLS - 1               # 8191 diffs per row
B = 4094                   # even sub-chunk boundary (keeps 16-bit APs 4B-aligned)
F32 = mybir.dt.float32
BF16 = mybir.dt.bfloat16
I16 = mybir.dt.int16


def build_nc(variant: str | None = None, reps: int = 1):
    """Build the per-core Bass program (same program for all 8 cores).

    reps>1 repeats the whole body (same inputs/outputs) for dispatch-
    overhead-cancelling wall-clock benchmarking: HW ~= (T_r - T_1)/(r-1).
    """
    nc = bass.Bass("TRN2", target_bir_lowering=False, debug=False)
    x = nc.dram_tensor("x", [SH_ROWS, COLS], F32, kind="ExternalInput").ap()
    iota = nc.dram_tensor("iota16", [P, D], I16, kind="ExternalInput").ap()
    y = nc.dram_tensor("y", [P, 2 * N_TILES], F32, kind="ExternalOutput").ap()

    H = COLS // 2  # DMA/nz chunk size
    sub_chunks = [(0, B), (B, D)]  # diff index ranges

    with tile.TileContext(nc) as tc:
        with (
            tc.tile_pool(name="const", bufs=1) as cpool,
            tc.tile_pool(name="xin", bufs=2) as xpool,
            tc.tile_pool(name="scr", bufs=3) as spool,
            tc.tile_pool(name="small", bufs=2) as smpool,
            tc.tile_pool(name="outp", bufs=1) as opool,
        ):
            it16 = cpool.tile([P, D], I16)
            nc.sync.dma_start(it16[:], iota[:, :])
            loss = opool.tile([P, 2 * N_TILES], F32)
            for t in range(N_TILES * reps):
                t = t % N_TILES
                rows = slice(t * P, (t + 1) * P)
                xt = xpool.tile([P, COLS], F32, tag="xt")
                rlh = smpool.tile([P, 2], F32, tag="rlh")
                nzj = spool.tile([P, COLS], BF16, tag="big")
                for c in range(2):
                    cs = slice(c * H, (c + 1) * H)
                    nc.sync.dma_start(xt[:, cs], x[rows, cs])
                    # rl chunk count: accum((x != 0) + 0)
                    nc.vector.tensor_scalar(
                        nzj[:, cs],
                        xt[:, cs],
                        0.0,
                        0.0,
                        mybir.AluOpType.not_equal,
                        mybir.AluOpType.add,
                        accum_out=rlh[:, c : c + 1],
                    )
                # rl_m1 = (rlh0 - 1) + rlh1
                rl_m1 = smpool.tile([P, 1], F32, tag="rl_m1")
                nc.vector.scalar_tensor_tensor(
                    rl_m1[:],
                    rlh[:, 0:1],
                    -1.0,
                    rlh[:, 1:2],
                    mybir.AluOpType.add,
                    mybir.AluOpType.add,
                )
                sbf = spool.tile([P, COLS], BF16, tag="big")
                abf = spool.tile([P, COLS], BF16, tag="big")
                rs = smpool.tile([P, 2], F32, tag="rs")
                for (j0, j1) in sub_chunks:
                    # d = x[:, j+1] - x[:, j] for j in [j0, j1)
                    nc.vector.tensor_tensor(
                        sbf[:, j0:j1],
                        xt[:, j0 + 1 : j1 + 1],
                        xt[:, j0:j1],
                        mybir.AluOpType.subtract,
                    )
                    nc.scalar.activation(
                        abf[:, j0:j1],
                        sbf[:, j0:j1],
                        mybir.ActivationFunctionType.Abs,
                    )
                # masked row-sum chunks: (iota < rl-1) * |d|, accum
                # (junk `out` written in-place over abf)
                for c, (j0, j1) in enumerate(sub_chunks):
                    nc.vector.scalar_tensor_tensor(
                        abf[:, j0:j1],
                        it16[:, j0:j1],
                        rl_m1[:],
                        abf[:, j0:j1],
                        mybir.AluOpType.is_lt,
                        mybir.AluOpType.mult,
                        accum_out=rs[:, c : c + 1],
                    )
                # stage per-tile partial sums + rl-1; division happens on host
                nc.vector.tensor_tensor(
                    loss[:, 2 * t : 2 * t + 1], rs[:, 0:1], rs[:, 1:2],
                    mybir.AluOpType.add,
                )
                nc.vector.tensor_scalar(
                    loss[:, 2 * t + 1 : 2 * t + 2], rl_m1[:], 1.0, None,
                    mybir.AluOpType.add,
                )
            # y[p, 2t] = rowsum, y[p, 2t+1] = rl
            nc.sync.dma_start(y[:, :], loss[:])
    bad = _audit_multi_waits(nc)
    if bad:
        raise RuntimeError(f"multi-wait instructions present: {bad}")
    return nc


_NC_CACHE: dict[str, object] = {}


def _get_nc(variant: str | None = None):
    key = variant or os.environ.get("CONSEC_VARIANT", "v2")
    if key not in _NC_CACHE:
        _NC_CACHE[key] = build_nc(key)
    return _NC_CACHE[key]


def _losses_from_y(y: np.ndarray) -> np.ndarray:
    """y [P, 2*N_TILES] -> per-row losses [SH_ROWS] (local row = t*P + p)."""
    y = y.reshape(P, N_TILES, 2)
    rs = y[:, :, 0].T.reshape(-1)   # [N_TILES*P] row-major by (t, p)
    rl = y[:, :, 1].T.reshape(-1)
    return (rs.astype(np.float32) / rl.astype(np.float32))


def _iota16() -> np.ndarray:
    return np.broadcast_to(
        np.arange(D, dtype=np.int16)[None, :], (P, D)
    ).copy()


def kernel(x: np.ndarray, **run_kwargs) -> np.ndarray:
    """Full-input entry point: x [4096, 8192] f32 -> scalar f32 loss."""
    x = np.ascontiguousarray(np.asarray(x, dtype=np.float32))
    assert x.shape == (ROWS, COLS)
    nc = _get_nc()
    it = _iota16()
    in_maps = [
        {"x": x[i * SH_ROWS : (i + 1) * SH_ROWS], "iota16": it}
        for i in range(N_CORES)
    ]
    res = run_bass_kernel_spmd(nc, in_maps, list(range(N_CORES)), **run_kwargs)
    losses = np.concatenate(
        [_losses_from_y(res.results[i]["y"]) for i in range(N_CORES)]
    )
    total = losses[1:].sum(dtype=np.float64) / float(ROWS)
    out = np.float32(total)
    if run_kwargs:
        kernel.last_results = res  # type: ignore[attr-defined]
    return out



# revision 3
# speedup vs baseline: 1.5704x; 1.5704x over previous
"""ConsecutiveLoss (L1) Trainium2 kernel.

Reference semantics (per full input x [4096, 8192] f32):
    rl[i]     = count_nonzero(x[i, :])
    per_row_i = sum_{j=0}^{8190} |x[i,j+1]-x[i,j]| * (j+1 < rl[i]) / rl[i]
    out       = sum_{i>=1} per_row_i / 4096

Sharding: 4096 rows split across 8 NeuronCores (512 rows each, 4 tiles of
128 rows). Each core computes, per row, the UNMASKED sum of |diffs| and the
exact nonzero count rl; the host divides, applies the (rare) mask fallback,
and does the final 4095-row reduction.

Per-core per-tile engine assignment (measured rates on HW):
  - DMA: two 2 MiB chunk loads (f32 tile [128, 8192])
  - DVE: tensor_tensor subtract f32->bf16 (1x, ~4.4us/chunk)
         + nonzero-indicator tensor_scalar 1-op f32->bf16 (2x_2p, ~2.2us/chunk)
         + direct not_equal+add accum on the last 1024 cols (1x, ~1.2us)
  - ACT: Abs+accum on the bf16 diffs (rowsum of |d|)
         + Identity+accum on the indicator cols [0:7168) (rowsum -> count)
  The indicator columns are split ACT/DVE (7168/1024) to balance engine time.

Rows where rl < 8192 (exact zeros in x -- none for randn inputs) get an
exact host-side recompute, so the unmasked device sum is always corrected
to the reference's masked semantics.

This walrus build accepts only ONE sync wait per ISA instruction; TileContext
emits multi-wait instructions (stage-1B consumers + the tail drain). Both are
patched below by splitting waits onto single-wait NoOp/Drain carriers.
"""

import os

import numpy as np

import concourse.bass as bass
import concourse.mybir as mybir
import concourse.tile as tile
from concourse.bass_utils import run_bass_kernel_spmd

# --- workaround: single-sync-wait-per-instruction walrus ---
_ORIG_DRAIN_AND_BARRIER = tile.TileContext._drain_and_barrier


def _split_drain_and_barrier(self, tick_clock, wait_clock):
    from concourse.tile import ScopedClock

    drain_inst = self.nc.sync.drain()
    wait_clock.add_sem_waits(
        drain_inst.ins, ScopedClock({None: tick_clock.global_clock})
    )
    si = drain_inst.ins.sync_info
    waits = list(si.on_wait) if si is not None and si.on_wait else []
    if len(waits) > 1:
        ups = list(si.on_update) if si.on_update else []
        drain_inst.ins.sync_info = mybir.SyncInfo(on_wait=[waits[0]], on_update=ups)
        for w in waits[1:]:
            extra = self.nc.sync.drain()
            extra.ins.sync_info = mybir.SyncInfo(on_wait=[w], on_update=[])

    self.nc.all_engine_barrier()
    assert self.sems is not None
    popped = self.nc._tile_sem_poison_stack.pop()
    assert popped is self._sem_poison
    self.nc.clear_and_free_semaphores(list(self.sems.allocated().values()))
    self.nc.all_engine_barrier()


tile.TileContext._drain_and_barrier = _split_drain_and_barrier

_ORIG_COMMIT = tile.TileContext._commit_instruction


def _split_commit(self, inst, lazy_reg_writes: bool = True):
    si = getattr(inst, "sync_info", None)
    if (
        si is not None
        and si.on_wait
        and len(si.on_wait) > 1
        and inst.engine != mybir.EngineType.Unassigned
    ):
        waits = list(si.on_wait)
        ups = list(si.on_update) if si.on_update else []
        for w in waits[:-1]:
            nop = mybir.InstNoOp(
                name=self.nc.get_next_instruction_name(),
                sync_info=mybir.SyncInfo(on_wait=[w], on_update=[]),
                bass_nofuse=True,
                engine=inst.engine,
                text_hint="wait_split",
            )
            _ORIG_COMMIT(self, nop, lazy_reg_writes=False)
        inst.sync_info = mybir.SyncInfo(on_wait=[waits[-1]], on_update=ups)
    return _ORIG_COMMIT(self, inst, lazy_reg_writes)


tile.TileContext._commit_instruction = _split_commit


def _audit_multi_waits(nc) -> list[str]:
    bad = []
    for name, ins in nc.inst_map.items():
        si = getattr(ins, "sync_info", None)
        if si is not None and si.on_wait and len(si.on_wait) > 1:
            bad.append(f"{type(ins).__name__} {name} {ins.engine} x{len(si.on_wait)}")
    return bad


N_CORES = 8
ROWS, COLS = 4096, 8192
SH_ROWS = ROWS // N_CORES  # 512 rows per core
P = 128                    # SBUF partitions
N_TILES = SH_ROWS // P     # 4 tiles per core
D = COLS - 1               # 8191 diffs per row
HA = COLS // 2             # 4096: DMA/compute chunk boundary
SA = HA - 2                # 4094: sub chunk A size (keeps bf16 APs 4B-aligned)
C_ACT = 7168               # indicator cols summed on ACT
C_CR = COLS - C_ACT        # 1024 cols counted directly on DVE
F32 = mybir.dt.float32
BF16 = mybir.dt.bfloat16


def build_nc(nz_mode: str = "device"):
    """Build the per-core Bass program (same program for all 8 cores).

    y layout [P, 4*N_TILES] f32, per tile t:
      col 4t+0: rowsum |d| over diffs [0, SA)
      col 4t+1: rowsum |d| over diffs [SA, D)
      col 4t+2: count_nonzero over cols [0, C_ACT)   (0 in host-nz mode)
      col 4t+3: count_nonzero over cols [C_ACT, COLS) (0 in host-nz mode)
    """
    nc = bass.Bass("TRN2", target_bir_lowering=False, debug=False)
    x = nc.dram_tensor("x", [SH_ROWS, COLS], F32, kind="ExternalInput").ap()
    y = nc.dram_tensor("y", [P, 4 * N_TILES], F32, kind="ExternalOutput").ap()

    A = mybir.AluOpType
    AF = mybir.ActivationFunctionType
    device_nz = nz_mode == "device"

    with tile.TileContext(nc) as tc:
        with (
            tc.tile_pool(name="xin", bufs=2) as xpool,
            tc.tile_pool(name="dif", bufs=2) as dpool,
            tc.tile_pool(name="ind", bufs=2) as ipool,
            tc.tile_pool(name="jnk", bufs=1) as jpool,
            tc.tile_pool(name="outp", bufs=1) as opool,
        ):
            acc = opool.tile([P, 4 * N_TILES], F32)
            if device_nz:
                crj = jpool.tile([P, C_CR], BF16)
            for t in range(N_TILES):
                rows = slice(t * P, (t + 1) * P)
                xt = xpool.tile([P, COLS], F32, tag="xt")
                d = dpool.tile([P, D], BF16, tag="d")
                if device_nz:
                    ind = ipool.tile([P, C_ACT], BF16, tag="ind")
                nc.sync.dma_start(xt[:, 0:HA], x[rows, 0:HA])
                nc.sync.dma_start(xt[:, HA:COLS], x[rows, HA:COLS])
                # DVE chunk A: d = x[j+1]-x[j] for j in [0, SA); indicator
                nc.vector.tensor_tensor(
                    d[:, 0:SA], xt[:, 1:SA + 1], xt[:, 0:SA], A.subtract
                )
                if device_nz:
                    nc.vector.tensor_scalar(
                        ind[:, 0:HA], xt[:, 0:HA], 0.0, None, A.not_equal
                    )
                # ACT: |d| rowsum chunk A
                nc.scalar.activation(
                    d[:, 0:SA], d[:, 0:SA], AF.Abs,
                    accum_out=acc[:, 4 * t:4 * t + 1],
                )
                # DVE chunk B
                nc.vector.tensor_tensor(
                    d[:, SA:D], xt[:, SA + 1:COLS], xt[:, SA:D], A.subtract
                )
                if device_nz:
                    nc.vector.tensor_scalar(
                        ind[:, HA:C_ACT], xt[:, HA:C_ACT], 0.0, None,
                        A.not_equal
                    )
                    # direct count of the tail columns (1x, small)
                    nc.vector.tensor_scalar(
                        crj[:], xt[:, C_ACT:COLS], 0.0, 0.0, A.not_equal,
                        A.add, accum_out=acc[:, 4 * t + 3:4 * t + 4],
                    )
                nc.scalar.activation(
                    d[:, SA:D], d[:, SA:D], AF.Abs,
                    accum_out=acc[:, 4 * t + 1:4 * t + 2],
                )
                if device_nz:
                    nc.scalar.activation(
                        ind[:, 0:C_ACT], ind[:, 0:C_ACT], AF.Identity,
                        accum_out=acc[:, 4 * t + 2:4 * t + 3],
                    )
                else:
                    nc.vector.memset(acc[:, 4 * t + 2:4 * t + 4], 0.0)
            nc.sync.dma_start(y[:, :], acc[:])
    bad = _audit_multi_waits(nc)
    if bad:
        raise RuntimeError(f"multi-wait instructions present: {bad}")
    return nc


_NC_CACHE: dict[str, object] = {}


def _get_nc(nz_mode: str | None = None):
    key = nz_mode or os.environ.get("CONSEC_NZ", "device")
    if key not in _NC_CACHE:
        _NC_CACHE[key] = build_nc(key)
    return _NC_CACHE[key]


def _np_row_loss_exact(row: np.ndarray) -> float:
    """Reference per-row loss (float64) for fallback rows."""
    rl = int(np.count_nonzero(row))
    if rl == 0:
        return float("nan")
    diffs = np.abs(np.diff(row.astype(np.float64)))
    pos = np.arange(1, row.shape[0])
    return float((diffs * (pos < rl)).sum() / rl)


def kernel(x: np.ndarray, **run_kwargs) -> np.ndarray:
    """Full-input entry point: x [4096, 8192] f32 -> scalar f32 loss."""
    x = np.ascontiguousarray(np.asarray(x, dtype=np.float32))
    assert x.shape == (ROWS, COLS)
    nz_mode = os.environ.get("CONSEC_NZ", "device")
    nc = _get_nc(nz_mode)
    in_maps = [
        {"x": x[i * SH_ROWS: (i + 1) * SH_ROWS]} for i in range(N_CORES)
    ]
    res = run_bass_kernel_spmd(nc, in_maps, list(range(N_CORES)), **run_kwargs)

    losses = np.empty(ROWS, dtype=np.float64)
    for i in range(N_CORES):
        yv = res.results[i]["y"].astype(np.float64)  # [P, 4*N_TILES]
        yv = yv.reshape(P, N_TILES, 4)
        rs = (yv[:, :, 0] + yv[:, :, 1]).T.reshape(-1)  # local row = t*P+p
        if nz_mode == "device":
            rl = (yv[:, :, 2] + yv[:, :, 3]).T.reshape(-1)
        else:
            shard = x[i * SH_ROWS: (i + 1) * SH_ROWS]
            rl = np.count_nonzero(shard, axis=1).astype(np.float64)
        losses[i * SH_ROWS: (i + 1) * SH_ROWS] = rs / np.maximum(rl, 1.0)
        # exact fallback for rows whose mask is not all-ones
        bad = np.where(rl < COLS)[0]
        for r in bad:
            losses[i * SH_ROWS + r] = _np_row_loss_exact(
                x[i * SH_ROWS + r]
            )
    total = losses[1:].sum() / float(ROWS)
    out = np.float32(total)
    if run_kwargs:
        kernel.last_results = res  # type: ignore[attr-defined]
    return out


# revision 7
# speedup vs baseline: 1.5725x; 1.0013x over previous
"""ConsecutiveLoss (L1) Trainium2 kernel.

Reference semantics (per full input x [4096, 8192] f32):
    rl[i]     = count_nonzero(x[i, :])
    per_row_i = sum_{j=0}^{8190} |x[i,j+1]-x[i,j]| * (j+1 < rl[i]) / rl[i]
    out       = sum_{i>=1} per_row_i / 4096

Sharding: 4096 rows split across 8 NeuronCores (512 rows each, 4 tiles of
128 rows). Each core computes, per row, the UNMASKED sum of |diffs| and the
exact nonzero count rl; the host divides, applies the (rare) mask fallback,
and does the final 4095-row reduction.

Per-core per-tile engine assignment (measured rates on HW):
  - DMA: two 2 MiB chunk loads (f32 tile [128, 8192])
  - DVE: tensor_tensor subtract f32->bf16 (1x, ~4.4us/chunk)
         + nonzero-indicator tensor_scalar 1-op f32->bf16 (2x_2p, ~2.2us/chunk)
         + direct not_equal+add accum on the last 1024 cols (1x, ~1.2us)
  - ACT: Abs+accum on the bf16 diffs (rowsum of |d|)
         + Identity+accum on the indicator cols [0:7168) (rowsum -> count)
  The indicator columns are split ACT/DVE (7168/1024) to balance engine time.

Rows where rl < 8192 (exact zeros in x -- none for randn inputs) get an
exact host-side recompute, so the unmasked device sum is always corrected
to the reference's masked semantics.

This walrus build accepts only ONE sync wait per ISA instruction; TileContext
emits multi-wait instructions (stage-1B consumers + the tail drain). Both are
patched below by splitting waits onto single-wait NoOp/Drain carriers.
"""

import os

import numpy as np

import concourse.bass as bass
import concourse.mybir as mybir
import concourse.tile as tile
from concourse.bass_utils import run_bass_kernel_spmd

# --- workaround: single-sync-wait-per-instruction walrus ---
_ORIG_DRAIN_AND_BARRIER = tile.TileContext._drain_and_barrier


def _split_drain_and_barrier(self, tick_clock, wait_clock):
    from concourse.tile import ScopedClock

    drain_inst = self.nc.sync.drain()
    wait_clock.add_sem_waits(
        drain_inst.ins, ScopedClock({None: tick_clock.global_clock})
    )
    si = drain_inst.ins.sync_info
    waits = list(si.on_wait) if si is not None and si.on_wait else []
    if len(waits) > 1:
        ups = list(si.on_update) if si.on_update else []
        drain_inst.ins.sync_info = mybir.SyncInfo(on_wait=[waits[0]], on_update=ups)
        for w in waits[1:]:
            extra = self.nc.sync.drain()
            extra.ins.sync_info = mybir.SyncInfo(on_wait=[w], on_update=[])

    self.nc.all_engine_barrier()
    assert self.sems is not None
    popped = self.nc._tile_sem_poison_stack.pop()
    assert popped is self._sem_poison
    self.nc.clear_and_free_semaphores(list(self.sems.allocated().values()))
    self.nc.all_engine_barrier()


tile.TileContext._drain_and_barrier = _split_drain_and_barrier

_ORIG_COMMIT = tile.TileContext._commit_instruction


def _split_commit(self, inst, lazy_reg_writes: bool = True):
    si = getattr(inst, "sync_info", None)
    if (
        si is not None
        and si.on_wait
        and len(si.on_wait) > 1
        and inst.engine != mybir.EngineType.Unassigned
    ):
        waits = list(si.on_wait)
        ups = list(si.on_update) if si.on_update else []
        for w in waits[:-1]:
            nop = mybir.InstNoOp(
                name=self.nc.get_next_instruction_name(),
                sync_info=mybir.SyncInfo(on_wait=[w], on_update=[]),
                bass_nofuse=True,
                engine=inst.engine,
                text_hint="wait_split",
            )
            _ORIG_COMMIT(self, nop, lazy_reg_writes=False)
        inst.sync_info = mybir.SyncInfo(on_wait=[waits[-1]], on_update=ups)
    return _ORIG_COMMIT(self, inst, lazy_reg_writes)


tile.TileContext._commit_instruction = _split_commit


def _audit_multi_waits(nc) -> list[str]:
    bad = []
    for name, ins in nc.inst_map.items():
        si = getattr(ins, "sync_info", None)
        if si is not None and si.on_wait and len(si.on_wait) > 1:
            bad.append(f"{type(ins).__name__} {name} {ins.engine} x{len(si.on_wait)}")
    return bad


N_CORES = 8
ROWS, COLS = 4096, 8192
SH_ROWS = ROWS // N_CORES  # 512 rows per core
P = 128                    # SBUF partitions
N_TILES = SH_ROWS // P     # 4 tiles per core
D = COLS - 1               # 8191 diffs per row
HA = COLS // 2             # 4096: DMA/compute chunk boundary
SA = HA - 2                # 4094: sub chunk A size (keeps bf16 APs 4B-aligned)
C_ACT = 7168               # indicator cols summed on ACT
C_CR = COLS - C_ACT        # 1024 cols counted directly on DVE
F32 = mybir.dt.float32
BF16 = mybir.dt.bfloat16


YC = 8  # output cols per tile: rs0..rs3, rlA0, rlA1, rlCR, pad


def build_nc(nz_mode: str = "device"):
    """Build the per-core Bass program (same program for all 8 cores).

    y layout [P, YC*N_TILES] f32, per tile t (unused cols stay 0):
      cols 8t+0..3: partial rowsums of |d| (host sums them)
      cols 8t+4..5: partial nonzero counts from the ACT indicator sums
      col  8t+6:    nonzero count of the tail cols (DVE direct reduce)

    The first and last tiles use finer DMA/compute chunks so the pipeline
    fills early and drains early; middle tiles use the balanced 2-chunk
    layout. Indicator columns [0, C_ACT) are summed on ACT, the rest are
    counted directly on DVE -- measured engine rates balance there.
    """
    nc = bass.Bass("TRN2", target_bir_lowering=False, debug=False)
    x = nc.dram_tensor("x", [SH_ROWS, COLS], F32, kind="ExternalInput").ap()
    y = nc.dram_tensor("y", [P, YC * N_TILES], F32, kind="ExternalOutput").ap()

    A = mybir.AluOpType
    AF = mybir.ActivationFunctionType
    device_nz = nz_mode == "device"
    Q = COLS // 4  # 2048: fine chunk

    with tile.TileContext(nc) as tc:
        with (
            tc.tile_pool(name="xin", bufs=2) as xpool,
            tc.tile_pool(name="dif", bufs=2) as dpool,
            tc.tile_pool(name="ind", bufs=2) as ipool,
            tc.tile_pool(name="jnk", bufs=1) as jpool,
            tc.tile_pool(name="outp", bufs=1) as opool,
        ):
            acc = opool.tile([P, YC * N_TILES], F32)
            nc.vector.memset(acc[:], 0.0)
            if device_nz:
                crj = jpool.tile([P, C_CR], BF16)

            def emit_tile(t, dma_splits, sub_splits, ident_splits):
                """dma_splits/sub_splits/ident_splits: col boundaries."""
                rows = slice(t * P, (t + 1) * P)
                xt = xpool.tile([P, COLS], F32, tag="xt")
                d = dpool.tile([P, D], BF16, tag="d")
                ind = (
                    ipool.tile([P, C_ACT], BF16, tag="ind", name="ind")
                    if device_nz else None
                )
                for a, b in zip(dma_splits[:-1], dma_splits[1:]):
                    nc.sync.dma_start(xt[:, a:b], x[rows, a:b])
                # DVE: subtract (diff range [a,b) reads x cols [a, b+1)),
                # interleaved with indicator creation over the same span.
                pend_ind = 0
                for k, (a, b) in enumerate(
                    zip(sub_splits[:-1], sub_splits[1:])
                ):
                    nc.vector.tensor_tensor(
                        d[:, a:b], xt[:, a + 1:b + 1], xt[:, a:b], A.subtract
                    )
                    # even chunk widths keep the 2x_2p DVE mode eligible
                    ie = min((b + 1) & ~1, C_ACT) if device_nz else 0
                    if device_nz and ie > pend_ind:
                        nc.vector.tensor_scalar(
                            ind[:, pend_ind:ie], xt[:, pend_ind:ie], 0.0,
                            None, A.not_equal
                        )
                        pend_ind = ie
                if device_nz:
                    if pend_ind < C_ACT:
                        nc.vector.tensor_scalar(
                            ind[:, pend_ind:C_ACT], xt[:, pend_ind:C_ACT],
                            0.0, None, A.not_equal
                        )
                    nc.vector.tensor_scalar(
                        crj[:], xt[:, C_ACT:COLS], 0.0, 0.0, A.not_equal,
                        A.add, accum_out=acc[:, YC * t + 6:YC * t + 7],
                    )
                # ACT: |d| rowsums per sub chunk + indicator sums, issued in
                # dependency-ready order (abs chunks as their subs finish,
                # ident chunks as their indicator spans finish).
                ident_jobs = list(zip(ident_splits[:-1], ident_splits[1:])) \
                    if device_nz else []
                next_ident = 0
                for k, (a, b) in enumerate(
                    zip(sub_splits[:-1], sub_splits[1:])
                ):
                    nc.scalar.activation(
                        d[:, a:b], d[:, a:b], AF.Abs,
                        accum_out=acc[:, YC * t + k:YC * t + k + 1],
                    )
                    while next_ident < len(ident_jobs) and \
                            ident_jobs[next_ident][1] <= b + 1:
                        ia, ib = ident_jobs[next_ident]
                        nc.scalar.activation(
                            ind[:, ia:ib], ind[:, ia:ib], AF.Identity,
                            accum_out=acc[:, YC * t + 4 + next_ident:
                                          YC * t + 5 + next_ident],
                        )
                        next_ident += 1
                for ia, ib in ident_jobs[next_ident:]:
                    nc.scalar.activation(
                        ind[:, ia:ib], ind[:, ia:ib], AF.Identity,
                        accum_out=acc[:, YC * t + 4 + next_ident:
                                      YC * t + 5 + next_ident],
                    )
                    next_ident += 1

            # tile 0: fine chunks everywhere (early pipeline fill)
            emit_tile(
                0,
                dma_splits=[0, Q, 2 * Q, 3 * Q, COLS],
                sub_splits=[0, Q - 2, 2 * Q - 2, 3 * Q - 2, D],
                ident_splits=[0, HA, C_ACT],
            )
            # middle tiles: balanced 2-chunk layout
            for t in range(1, N_TILES - 1):
                emit_tile(
                    t,
                    dma_splits=[0, HA, COLS],
                    sub_splits=[0, SA, D],
                    ident_splits=[0, C_ACT],
                )
            # last tile: fine chunks on the B half (early pipeline drain)
            emit_tile(
                N_TILES - 1,
                dma_splits=[0, HA, 3 * Q, COLS],
                sub_splits=[0, SA, 3 * Q - 2, D],
                ident_splits=[0, HA, C_ACT],
            )
            nc.sync.dma_start(y[:, :], acc[:])
    bad = _audit_multi_waits(nc)
    if bad:
        raise RuntimeError(f"multi-wait instructions present: {bad}")
    return nc


_NC_CACHE: dict[str, object] = {}


def _get_nc(nz_mode: str | None = None):
    key = nz_mode or os.environ.get("CONSEC_NZ", "device")
    if key not in _NC_CACHE:
        _NC_CACHE[key] = build_nc(key)
    return _NC_CACHE[key]


def _np_row_loss_exact(row: np.ndarray) -> float:
    """Reference per-row loss (float64) for fallback rows."""
    rl = int(np.count_nonzero(row))
    if rl == 0:
        return float("nan")
    diffs = np.abs(np.diff(row.astype(np.float64)))
    pos = np.arange(1, row.shape[0])
    return float((diffs * (pos < rl)).sum() / rl)


def kernel(x: np.ndarray, **run_kwargs) -> np.ndarray:
    """Full-input entry point: x [4096, 8192] f32 -> scalar f32 loss."""
    x = np.ascontiguousarray(np.asarray(x, dtype=np.float32))
    assert x.shape == (ROWS, COLS)
    nz_mode = os.environ.get("CONSEC_NZ", "device")
    nc = _get_nc(nz_mode)
    in_maps = [
        {"x": x[i * SH_ROWS: (i + 1) * SH_ROWS]} for i in range(N_CORES)
    ]
    res = run_bass_kernel_spmd(nc, in_maps, list(range(N_CORES)), **run_kwargs)

    losses = np.empty(ROWS, dtype=np.float64)
    for i in range(N_CORES):
        yv = res.results[i]["y"].astype(np.float64)  # [P, YC*N_TILES]
        yv = yv.reshape(P, N_TILES, YC)
        rs = yv[:, :, 0:4].sum(axis=2).T.reshape(-1)  # local row = t*P+p
        if nz_mode == "device":
            rl = yv[:, :, 4:7].sum(axis=2).T.reshape(-1)
        else:
            shard = x[i * SH_ROWS: (i + 1) * SH_ROWS]
            rl = np.count_nonzero(shard, axis=1).astype(np.float64)
        losses[i * SH_ROWS: (i + 1) * SH_ROWS] = rs / np.maximum(rl, 1.0)
        # exact fallback for rows whose mask is not all-ones
        bad = np.where(rl < COLS)[0]
        for r in bad:
            losses[i * SH_ROWS + r] = _np_row_loss_exact(
                x[i * SH_ROWS + r]
            )
    total = losses[1:].sum() / float(ROWS)
    out = np.float32(total)
    if run_kwargs:
        kernel.last_results = res  # type: ignore[attr-defined]
    return out
